# revision 19
# baseline (speedup 1.0000x reference)
"""Trainium2 Bass kernel for the controlled-U (CU) gate application.

Math: the reference builds U = P0 (x) I (x) ... + P1 (x) Mexp (x) I ...
with dim=2, wires=12, index=(0,1), control_state=(1,). This factors as

    U = diag(I_2048, Mexp (x) I_1024)        (4096 x 4096)

so U @ x is:
    out[0:2048]     = x[0:2048]                        (identity)
    out[2048:3072]  = c00 * x[2048:3072] + c01 * x[3072:4096]
    out[3072:4096]  = c10 * x[2048:3072] + c11 * x[3072:4096]

with [[c00, c01], [c10, c11]] = Mexp = expm(M - M^H), a 2x2 unitary
computed exactly on host (eigendecomposition of the 2x2 Hermitian
generator).

The kernel is pure data movement + a broadcast 2x2 mix, so it is HBM
bandwidth bound.  The device stages everything in bf16 (the harness
gate is rel_err < 2e-2; the bf16 pipeline measures ~2.5e-3), which
halves DMA traffic to ~4.2 MiB per core vs f32 staging.  All device
tensors are PLANAR [re | im] pairs; the host interleaves to complex64
while it upcasts during the gather (it touches every element there
anyway), so no engine pays the strided-write interleave penalty.

  - 8 cores, SPMD row sharding: core d gets top rows [256d, 256d+256)
    (identity) plus the bottom pair rows [2048+128d, +128) and
    [3072+128d, +128) (the 2x2 mix).
  - top rows are a pure passthrough: two HBM->HBM DMAs on the ACT
    HWDGE ring, issued first -- they stream while the bottom tiles
    load on the SP ring, with zero compute and zero SBUF.
  - bottom mix = 4 output quantities, each sum_k coef_k * in_k:
      o1 -> DVE: 1 tensor_scalar + 3 fused scalar_tensor_tensor MACs
        in bf16, writing the planar halves of out_b1 densely.
      o2 -> TensorE: bf16 matmuls with c_k * I_128 diagonal
        stationaries, 512-column moving halves (one PSUM bank per
        matmul), f32 PSUM accumulation over the 4 inputs; ACT copies
        PSUM -> SBUF densely (bf16 cast).
  - out_b1 stores ride the SP ring after the loads (per quantity, so
    the first store issues as soon as the o1re chain finishes);
    out_b2 stores ride the ACT ring right after its evacuations.
"""

import ml_dtypes
import numpy as np

import concourse.bacc as bacc
import concourse.mybir as mybir
from concourse.tile import TileContext
from concourse.bass_utils import run_bass_kernel_spmd

# Problem geometry (hardcoded per the task contract).
D = 4096           # state dimension 2**12
B = 1024           # batch
NCORES = 8
P = 128            # SBUF partitions
TROWS = D // 2 // NCORES   # 256 top (identity) rows per core
PROWS = D // 4 // NCORES   # 128 bottom pair rows per core
CH = B // 2        # column half (PSUM bank = 512 f32)
F32 = mybir.dt.float32
BF16 = mybir.dt.bfloat16
NPBF = ml_dtypes.bfloat16

NDIAG = 12         # 12 coefficient scalars (signs baked in)
NPED = 6           # distinct coefficients used by the PE (o2) quantities

# quantity -> (planar half, coefficient idx per input).
# inputs are (xr1, xi1, xr2, xi2); coefficients include baked-in signs.
RECIPES = [
    ("o1re", 0, (0, 1, 3, 4)),
    ("o1im", 1, (2, 0, 5, 3)),
    ("o2re", 0, (6, 7, 9, 10)),
    ("o2im", 1, (8, 6, 11, 9)),
]


def _build_nc() -> bacc.Bacc:
    """Build the per-core Bass/Tile program (identical on all 8 cores)."""
    nc = bacc.Bacc("TRN2", enable_partition_id=False)

    # column-concatenated bf16 input pairs (packed on host)
    xb_a = nc.dram_tensor("xb_a", [PROWS, 2 * B], BF16, kind="ExternalInput")
    xb_b = nc.dram_tensor("xb_b", [PROWS, 2 * B], BF16, kind="ExternalInput")
    xt0 = nc.dram_tensor("xt0", [P, 2 * B], BF16, kind="ExternalInput")
    xt1 = nc.dram_tensor("xt1", [P, 2 * B], BF16, kind="ExternalInput")
    # coef[p, j*P + p] = value_{6+j}  ->  128x128 diagonal blocks, plus
    # the 12 f32 DVE scalars appended as 24 bf16-slot raw bytes (avoids a
    # separate tiny-descriptor cvec DMA at the head of the ring).
    coef = nc.dram_tensor("coef", [P, NPED * P + 2 * NDIAG], BF16,
                          kind="ExternalInput")

    out_t = nc.dram_tensor("out_t", [TROWS, 2 * B], BF16,
                           kind="ExternalOutput")
    out_b1 = nc.dram_tensor("out_b1", [PROWS, 2 * B], BF16,
                            kind="ExternalOutput")
    out_b2 = nc.dram_tensor("out_b2", [PROWS, 2 * B], BF16,
                            kind="ExternalOutput")

    mul = mybir.AluOpType.mult
    add = mybir.AluOpType.add

    with TileContext(nc) as tc:
        with (
            tc.tile_pool(name="const", bufs=1) as const_pool,
            tc.tile_pool(name="io", bufs=1) as io_pool,
            tc.tile_pool(name="scr", bufs=1) as scr_pool,
            tc.tile_pool(name="psum", bufs=1, space="PSUM") as psum_pool,
        ):
            coef_sb = const_pool.tile([P, NPED * P + 2 * NDIAG], BF16)
            cvec_sb = coef_sb[:, NPED * P : NPED * P + 2 * NDIAG].bitcast(F32)

            def cval(k: int):
                """value_k as a per-partition scalar operand."""
                return cvec_sb[:, k : k + 1]

            def cdiag(k: int):
                """value_k * I_128 stationary (k is a RECIPES coef index)."""
                j = k - 6
                return coef_sb[:, j * P : (j + 1) * P]

            # ---- loads: coef + bottom pair rows on the SP ring, then the
            # top-half HBM->HBM passthrough QUEUED BEHIND them (strict ring
            # FIFO = bottom data streams first, passthrough fills the HBM
            # while the engines compute; no SBUF, no load->store sem hop).
            ba = io_pool.tile([P, 2 * B], BF16, tag="ba")
            bb = io_pool.tile([P, 2 * B], BF16, tag="bb")
            nc.sync.dma_start(coef_sb[:], coef[:])
            # bottom tiles split per component (256 KiB each) so the first
            # DVE products and PE matmuls start as early as possible
            nc.sync.dma_start(ba[:, 0:B], xb_a[:, 0:B])
            nc.sync.dma_start(ba[:, B : 2 * B], xb_a[:, B : 2 * B])
            nc.sync.dma_start(bb[:, 0:B], xb_b[:, 0:B])
            nc.sync.dma_start(bb[:, B : 2 * B], xb_b[:, B : 2 * B])
            nc.sync.dma_start(out_t[0:P, :], xt0[:])
            nc.sync.dma_start(out_t[P : 2 * P, :], xt1[:])

            b_in = {
                "r1": ba[:, 0:B], "i1": ba[:, B : 2 * B],
                "r2": bb[:, 0:B], "i2": bb[:, B : 2 * B],
            }

            # Engine warmups.  The PE gets a stream of dummy matmuls on a
            # memset tile: the HAM clock gate needs ~3.4 us of sustained PE
            # activity before it unthrottles 1.2 -> 2.4 GHz, and the dummy
            # stream bridges the gap until the payload inputs land so the
            # payload matmuls run warm (~216 ns vs ~630 ns each).
            dummy = scr_pool.tile([P, CH], BF16, tag="dummy")
            nc.gpsimd.memset(dummy[:], 0.0)
            warm_ps = psum_pool.tile([P, CH], F32, tag="warm")
            for _ in range(10):
                nc.tensor.matmul(warm_ps[:], dummy[:, 0:P], dummy[:],
                                 start=True, stop=True)
            warm_v = scr_pool.tile([P, 2], F32, tag="warm_v")
            nc.vector.tensor_copy(warm_v[:], cvec_sb[:, 0:2])
            warm_a = scr_pool.tile([P, 2], F32, tag="warm_a")
            nc.scalar.copy(warm_a[:], cvec_sb[:, 0:2])

            o_b1 = io_pool.tile([P, 2 * B], BF16, tag="o_b1")
            o_b2 = io_pool.tile([P, 2 * B], BF16, tag="o_b2")

            # PE: o2re then o2im, per 512-column half (one PSUM bank per
            # matmul output), 16 bf16 matmuls total
            pts = {}
            for c in range(2):
                cs = slice(c * CH, (c + 1) * CH)
                for name, half, cks in RECIPES[2:]:
                    pt = psum_pool.tile([P, CH], F32, tag=f"ps{half}_{c}",
                                        name=f"ps{half}_{c}")
                    for t, (k, nm) in enumerate(
                            zip(cks, ("r1", "i1", "r2", "i2"))):
                        nc.tensor.matmul(pt[:], cdiag(k), b_in[nm][:, cs],
                                         start=(t == 0), stop=(t == 3))
                    pts[(half, c)] = pt

            # DVE: o1re, o1im as 4 tensor_scalar products + 3 tensor_tensor
            # adds each -- scalar_tensor_tensor has no 2x uop (1x, ~1.28 us
            # per op) while TS/TT bf16 dense run at 4x/2x (~0.49/0.59 us).
            # Products are emitted in load-arrival order (r1, i1, r2, i2)
            # across BOTH quantities, with the partial adds as soon as
            # their terms exist, so the DVE never waits on late tiles.
            s = {}
            for t, nm in enumerate(("r1", "i1", "r2", "i2")):
                for name, half, cks in RECIPES[:2]:
                    st = scr_pool.tile([P, B], BF16, tag=f"s{half}_{t}",
                                       name=f"s{half}_{t}")
                    nc.vector.tensor_scalar_mul(st[:], b_in[nm],
                                                cval(cks[t]))
                    s[(half, t)] = st
                if t == 1:
                    for half in (0, 1):
                        nc.vector.tensor_add(
                            s[(half, 0)][:], s[(half, 0)][:],
                            s[(half, 1)][:])
            for name, half, cks in RECIPES[:2]:
                nc.vector.tensor_add(
                    s[(half, 2)][:], s[(half, 2)][:], s[(half, 3)][:])
                nc.vector.tensor_add(
                    o_b1[:, half * B : (half + 1) * B],
                    s[(half, 0)][:], s[(half, 2)][:])
                # o1 store per quantity on the ACT ring (the SP ring's
                # FIFO is busy with the top passthrough)
                nc.scalar.dma_start(
                    out_b1[:, half * B : (half + 1) * B],
                    o_b1[:, half * B : (half + 1) * B])

            # ACT: dense PSUM -> SBUF evacuations (bf16 cast), then the
            # out_b2 store on the same ring (FIFO, no sem hop).
            for (half, c), pt in pts.items():
                ds = slice(half * B + c * CH, half * B + (c + 1) * CH)
                nc.scalar.copy(o_b2[:, ds], pt[:])
            nc.scalar.dma_start(out_b2[:], o_b2[:])

    nc.finalize()
    return nc


_NC_CACHE = None


def _get_nc() -> bacc.Bacc:
    global _NC_CACHE
    if _NC_CACHE is None:
        _NC_CACHE = _build_nc()
    return _NC_CACHE


def _coef_values(M_re: np.ndarray, M_im: np.ndarray):
    """Host-side 2x2 expm of the anti-Hermitian generator -> coef arrays."""
    M = M_re.astype(np.float64) + 1j * M_im.astype(np.float64)
    A = M - M.conj().T          # anti-Hermitian
    H = -1j * A                 # Hermitian
    w, V = np.linalg.eigh(H)
    Mexp = V @ np.diag(np.exp(1j * w)) @ V.conj().T   # expm(A), exact
    c00, c01 = Mexp[0, 0], Mexp[0, 1]
    c10, c11 = Mexp[1, 0], Mexp[1, 1]
    vals = [
        c00.real, -c00.imag, c00.imag,
        c01.real, -c01.imag, c01.imag,
        c10.real, -c10.imag, c10.imag,
        c11.real, -c11.imag, c11.imag,
    ]
    coef = np.zeros((P, NPED * P + 2 * NDIAG), dtype=NPBF)
    idx = np.arange(P)
    for j in range(NPED):
        coef[idx, j * P + idx] = NPBF(vals[6 + j])
    # the 12 f32 DVE scalars ride along as raw bytes in the last 24 slots
    cvec = np.tile(np.array(vals, dtype=np.float32), (P, 1))
    coef[:, NPED * P :] = cvec.view(np.uint16).view(NPBF)
    return coef, cvec


def _in_map(x_re, x_im, coef, cvec, d: int) -> dict:
    """Per-core input dict; casts the core's slices to bf16 and packs
    column-concatenated [re | im] pairs."""
    t0 = d * TROWS
    b1 = D // 2 + d * PROWS
    b2 = 3 * D // 4 + d * PROWS

    def bf(a):
        return np.ascontiguousarray(a).astype(NPBF)

    return {
        "xb_a": np.concatenate(
            [bf(x_re[b1 : b1 + PROWS]), bf(x_im[b1 : b1 + PROWS])], axis=1),
        "xb_b": np.concatenate(
            [bf(x_re[b2 : b2 + PROWS]), bf(x_im[b2 : b2 + PROWS])], axis=1),
        "xt0": np.concatenate(
            [bf(x_re[t0 : t0 + P]), bf(x_im[t0 : t0 + P])], axis=1),
        "xt1": np.concatenate(
            [bf(x_re[t0 + P : t0 + TROWS]), bf(x_im[t0 + P : t0 + TROWS])],
            axis=1),
        "coef": coef,
    }


def kernel(M_re, M_im, x_re, x_im) -> np.ndarray:
    M_re = np.asarray(M_re, dtype=np.float32)
    M_im = np.asarray(M_im, dtype=np.float32)
    x_re = np.ascontiguousarray(x_re, dtype=np.float32)
    x_im = np.ascontiguousarray(x_im, dtype=np.float32)

    coef, cvec = _coef_values(M_re, M_im)
    in_maps = [_in_map(x_re, x_im, coef, cvec, d) for d in range(NCORES)]

    nc = _get_nc()
    res = run_bass_kernel_spmd(nc, in_maps, core_ids=list(range(NCORES)))

    full = np.empty((D, B), dtype=np.complex64)
    for d, r in enumerate(res.results):
        t0 = d * TROWS
        b1 = D // 2 + d * PROWS
        b2 = 3 * D // 4 + d * PROWS
        ot = np.asarray(r["out_t"])
        ob1 = np.asarray(r["out_b1"])
        ob2 = np.asarray(r["out_b2"])
        full.real[t0 : t0 + P] = ot[0:P, 0:B].astype(np.float32)
        full.imag[t0 : t0 + P] = ot[0:P, B : 2 * B].astype(np.float32)
        full.real[t0 + P : t0 + TROWS] = ot[P:, 0:B].astype(np.float32)
        full.imag[t0 + P : t0 + TROWS] = ot[P:, B : 2 * B].astype(np.float32)
        full.real[b1 : b1 + PROWS] = ob1[:, 0:B].astype(np.float32)
        full.imag[b1 : b1 + PROWS] = ob1[:, B : 2 * B].astype(np.float32)
        full.real[b2 : b2 + PROWS] = ob2[:, 0:B].astype(np.float32)
        full.imag[b2 : b2 + PROWS] = ob2[:, B : 2 * B].astype(np.float32)
    return full
